# revision 21
# baseline (speedup 1.0000x reference)
"""Causal multi-head attention block (B=4, S=2048, NX=1024, H=16, D=64)
distributed over 8 TRN2 NeuronCores.

Sharding: core i handles batch b = i//2 and head-group hg = i%2 (8 of 16
heads).  Each core computes qkv for its heads, causal attention, and a
partial c_proj over its 512 feature rows; the per-batch pair of partials
is summed on the host while unsharding.

All matmuls run in bf16 (f32 PSUM accumulate).  Scores are computed in the
transposed orientation s^T[k, q] = k @ q^T as concurrent 64-row quadrant
pairs (head A rows 0:63, head B rows 64:127 -> full moving-bus rate).  The
u = p @ v stage runs transposed with a v|ones augmented stationary so the
softmax denominator accumulates in the same pass.

Schedule: dense fill units (qk chunks / v tiles / proj tiles) are emitted
BEFORE each scores group so the in-order PE never head-of-line blocks on
the exp (scalar) engine draining score PSUM buffers.  Input DMAs are
sliced so the first qk chunk depends on ~1.3 MB instead of 5 MB.
"""
import sys

sys.path.insert(0, "/opt/trn_rl_repo")

import functools

import ml_dtypes
import numpy as np

from concourse import bacc, mybir, tile
from concourse.bass_utils import run_bass_kernel_spmd

B, S, NX = 4, 2048, 1024
H, D = 16, 64
N_CORES = 8
HL = H // 2          # heads per core (local)
FL = HL * D          # local head feature width (512)
BF16 = mybir.dt.bfloat16
F32 = mybir.dt.float32
BF = ml_dtypes.bfloat16

NK = S // 128        # 16 k-tiles of 128
KK = NX // 128       # 8 contraction blocks
FB = FL // 128       # 4 feature blocks per q/k half

DEFAULT_CFG = "host-psw1024-psb3-pb8-pub2-nb"
DEFAULT_CFG_BIAS = "host-psw1024-psb3-pb8-pub2"


def _parse_cfg(cfg: str):
    parts = cfg.split("-")
    d = {"mode": parts[0], "psw": 1024, "psb": 3, "pb": 8, "pub": 2,
         "nb": False, "gtri": False, "dpk": False}
    for p in parts[1:]:
        if p.startswith("psw"):
            d["psw"] = int(p[3:])
        elif p.startswith("psb"):
            d["psb"] = int(p[3:])
        elif p.startswith("pub"):
            d["pub"] = int(p[3:])
        elif p.startswith("pb"):
            d["pb"] = int(p[2:])
        elif p in d:
            d[p] = True
    return d


def _build(cfg: str):
    c = _parse_cfg(cfg)
    PSW, PSB, PB, PUB = c["psw"], c["psb"], c["pb"], c["pub"]
    NB = c["nb"]
    GTRI = c["gtri"]     # move tri-mask muls to gpsimd
    DPK = c["dpk"]       # disable packed pair reciprocal
    GK = PSW // 512      # full k-tiles per exp group
    nc = bacc.Bacc("TRN2", target_bir_lowering=False, debug=False,
                   num_devices=N_CORES)

    xT_ext = nc.dram_tensor("x8", [8, NX, 256], BF16, kind="ExternalInput")
    wqk_ext = nc.dram_tensor("w_qk8", [2 * FB, NX, 128], BF16,
                             kind="ExternalInput")
    wv_ext = nc.dram_tensor("w_v", [NX, FL], BF16, kind="ExternalInput")
    wp_ext = nc.dram_tensor("w_proj", [FL, NX], BF16, kind="ExternalInput")
    bqk_ext = nc.dram_tensor("b_qk", [2 * FL, 1], F32, kind="ExternalInput")
    bv_ext = nc.dram_tensor("bv_row", [1, FL], BF16, kind="ExternalInput")
    bp_ext = nc.dram_tensor("bp_row", [1, NX], BF16, kind="ExternalInput")
    out_ext = nc.dram_tensor("out", [S, NX], BF16, kind="ExternalOutput")

    with tile.TileContext(nc) as tc:
        with tc.tile_pool(name="const", bufs=1) as cp, \
             tc.tile_pool(name="work", bufs=3) as wp, \
             tc.tile_pool(name="psS", bufs=PSB, space="PSUM") as psS, \
             tc.tile_pool(name="psU", bufs=PUB, space="PSUM") as psU:

            # ---- persistent SBUF tensors ----
            xT = cp.tile([128, KK, S], BF16, tag="xT")
            wqk = cp.tile([128, KK, 2 * FL], BF16, tag="wqk")
            wv = cp.tile([128, KK, FL], BF16, tag="wv")
            wproj = cp.tile([128, FL // 128, NX], BF16, tag="wproj")
            qkT = cp.tile([128, 2 * FL // 128, S], BF16, tag="qkT")
            # v_aug[k, kt, hh, par, 0:128]: per head pair hh, parity par:
            #   par=0 (even head): cols 0:64 = v, 64:128 = 1.0
            #   par=1 (odd head):  cols 0:64 = 1.0, 64:128 = v
            v5 = cp.tile([128, NK, HL // 2, 2, 128], BF16, tag="v5")
            aT = cp.tile([128, FL // 128, S], BF16, tag="aT")  # a^T [feat, q]
            bqk = cp.tile([128, 2 * FL // 128], F32, tag="bqk")
            bv_row = cp.tile([1, FL], BF16, tag="bv")
            bp_row = cp.tile([1, NX], BF16, tag="bp")
            ones_row = cp.tile([1, 128], BF16, tag="ones")
            tri = cp.tile([128, 128], BF16, tag="tri")

            # ---- input DMAs: priority-ordered contiguous slices.
            # Never on the scalar ring (DGE would steal exp-engine time).
            xs = [xT_ext.ap()[j].rearrange("(kk p) c -> p kk c", p=128)
                  for j in range(8)]
            wqk8 = [wqk_ext.ap()[fb].rearrange("(kk p) c -> p kk c", p=128)
                    for fb in range(2 * FB)]

            def dma_x(ring, j):
                ring.dma_start(out=xT[:, :, j * 256:(j + 1) * 256],
                               in_=xs[j])

            def dma_wqk(ring, fb):
                ring.dma_start(out=wqk[:, :, fb * 128:(fb + 1) * 128],
                               in_=wqk8[fb])

            dma_x(nc.sync, 0)
            dma_wqk(nc.gpsimd, 0)
            dma_wqk(nc.gpsimd, FB)
            dma_x(nc.sync, 1)
            nc.sync.dma_start(
                out=bqk[:, :],
                in_=bqk_ext.ap().rearrange("(fb p) o -> p (fb o)", p=128))
            nc.sync.dma_start(out=bv_row[:], in_=bv_ext.ap())
            nc.sync.dma_start(out=bp_row[:], in_=bp_ext.ap())
            nc.gpsimd.dma_start(
                out=wv[:, :, :],
                in_=wv_ext.ap().rearrange("(kk p) f -> p kk f", p=128))
            dma_x(nc.sync, 2)
            dma_x(nc.sync, 3)
            dma_wqk(nc.gpsimd, 1)
            dma_wqk(nc.gpsimd, FB + 1)
            dma_x(nc.sync, 4)
            dma_x(nc.sync, 5)
            dma_wqk(nc.gpsimd, 2)
            dma_wqk(nc.gpsimd, FB + 2)
            dma_x(nc.sync, 6)
            dma_x(nc.sync, 7)
            dma_wqk(nc.gpsimd, 3)
            dma_wqk(nc.gpsimd, FB + 3)
            nc.sync.dma_start(
                out=wproj[:, :, :],
                in_=wp_ext.ap().rearrange("(kt p) f -> p kt f", p=128))

            nc.vector.memset(ones_row[:], 1.0)
            # tri[p, f] = 1 if p <= f else 0 (keep-in on p > f, else fill 1)
            nc.vector.memset(tri[:], 0.0)
            nc.gpsimd.affine_select(
                out=tri[:], in_=tri[:],
                compare_op=mybir.AluOpType.is_gt,
                fill=1.0, base=0, pattern=[[-1, 128]], channel_multiplier=1,
            )
            # ones halves of v_aug (parity-swapped)
            nc.vector.memset(v5[:, :, :, 0, 64:128], 1.0)
            nc.vector.memset(v5[:, :, :, 1, 0:64], 1.0)

            # ---- stage 2: v (natural layout, split by head parity) ----
            def emit_v(st):
                ps = psS.tile([128, FL], F32, tag="ps")
                for kk in range(KK):
                    nc.tensor.matmul(ps[:], xT[:, kk, st * 128:(st + 1) * 128],
                                     wv[:, kk, :], start=(kk == 0),
                                     stop=(NB and kk == KK - 1))
                if not NB:
                    nc.tensor.matmul(ps[:], ones_row[:], bv_row[:],
                                     start=False, stop=True)
                ps_r = ps[:].rearrange("p (hh par d) -> p hh par d",
                                       par=2, d=D)
                nc.vector.tensor_copy(v5[:, st, :, 0, 0:D], ps_r[:, :, 0, :])
                nc.vector.tensor_copy(v5[:, st, :, 1, D:128], ps_r[:, :, 1, :])

            # ---- stage 1: q^T / k^T chunks (feature-major, 512 cols) ----
            def emit_qk(fb, c0):
                ps = psS.tile([128, 512], F32, tag="ps")
                for kk in range(KK):
                    nc.tensor.matmul(
                        ps[:], wqk[:, kk, fb * 128:(fb + 1) * 128],
                        xT[:, kk, c0:c0 + 512],
                        start=(kk == 0), stop=(kk == KK - 1))
                nc.vector.tensor_scalar_add(qkT[:, fb, c0:c0 + 512],
                                            ps[:], bqk[:, fb:fb + 1])

            # ---- stage 3: attention ----
            def head_ctx(lh, qc):
                # Full k-tiles: one tile per group, both heads in planes of
                # one [128, 2, 512] PSUM tile (identical deps -> the quadrant
                # pair co-dispatches).  Diagonal k-tiles: two 2-tile groups
                # per head (old layout, contiguous exp reads).
                n_full = 4 * qc
                groups = [("f", kt) for kt in range(n_full)]
                groups.append(("d", [(n_full + 0, 0, 512, 0),
                                     (n_full + 1, 512, 384, 128)]))
                groups.append(("d", [(n_full + 2, 0, 256, 256),
                                     (n_full + 3, 256, 128, 384)]))
                return {"lh": lh, "fbq": lh // 2, "fbk": FB + lh // 2,
                        "po": (lh % 2) * 64, "qb": qc * 512, "qc": qc,
                        "n_full": n_full, "groups": groups,
                        "p": [None] * len(groups), "pu": None,
                        "last_kt": n_full + 3}

            def emit_scores_pair(A, Bc, gi):
                kind, g = A["groups"][gi]
                amv0 = A["qb"]
                bmv0 = Bc["qb"]
                if kind == "f":
                    kt = g
                    ps = psS.tile([128, 2, 512], F32, tag="ps", name="psP")
                    p = wp.tile([128, 2, 512], BF16, tag="p", bufs=PB,
                                name="pP")
                    k0 = kt * 128
                    nc.tensor.matmul(ps[:, 0, :],
                                     qkT[0:64, A["fbk"], k0:k0 + 128],
                                     qkT[0:64, A["fbq"], amv0:amv0 + 512],
                                     start=True, stop=True)
                    nc.tensor.matmul(ps[:, 1, :],
                                     qkT[64:128, Bc["fbk"], k0:k0 + 128],
                                     qkT[64:128, Bc["fbq"], bmv0:bmv0 + 512],
                                     start=True, stop=True)
                    nc.scalar.activation(p[:, :, :], ps[:, :, :],
                                         mybir.ActivationFunctionType.Exp,
                                         scale=0.125)
                    A["p"][gi] = p
                    Bc["p"][gi] = p
                    return
                gw = max(off + N for (_, off, N, _) in g)
                psA = psS.tile([128, 1024], F32, tag="ps", name="psA")
                psB = psS.tile([128, 1024], F32, tag="ps", name="psB")
                pA = wp.tile([128, 1024], BF16, tag="p", bufs=PB, name="pA")
                pB = wp.tile([128, 1024], BF16, tag="p", bufs=PB, name="pB")
                for (kt, off, N, qoff) in g:
                    k0 = kt * 128
                    nc.tensor.matmul(psA[:, off:off + N],
                                     qkT[0:64, A["fbk"], k0:k0 + 128],
                                     qkT[0:64, A["fbq"],
                                         amv0 + qoff:amv0 + 512],
                                     start=True, stop=True)
                    nc.tensor.matmul(psB[:, off:off + N],
                                     qkT[64:128, Bc["fbk"], k0:k0 + 128],
                                     qkT[64:128, Bc["fbq"],
                                         bmv0 + qoff:bmv0 + 512],
                                     start=True, stop=True)
                for ctx, ps_, p_ in ((A, psA, pA), (Bc, psB, pB)):
                    nc.scalar.activation(p_[:, 0:gw], ps_[:, 0:gw],
                                         mybir.ActivationFunctionType.Exp,
                                         scale=0.125)
                    eng = nc.gpsimd if GTRI else nc.vector
                    for (kt, off, N, qoff) in g:
                        eng.tensor_mul(p_[:, off:off + 128],
                                       p_[:, off:off + 128], tri)
                    ctx["p"][gi] = p_

            def emit_u(ctx, gi):
                if ctx["pu"] is None:
                    ctx["pu"] = psU.tile([128, 512], F32, tag="pu",
                                         name="pu_t")
                pu = ctx["pu"]
                p = ctx["p"][gi]
                lh = ctx["lh"]
                kind, g = ctx["groups"][gi]
                if kind == "f":
                    kt = g
                    nc.tensor.matmul(
                        pu[:, 0:512],
                        v5[:, kt, lh >> 1, lh & 1, :],
                        p[:, lh & 1, :],
                        start=(kt == 0), stop=(kt == ctx["last_kt"]),
                        skip_group_check=True)
                    return
                for (kt, off, N, qoff) in g:
                    nc.tensor.matmul(
                        pu[:, qoff:qoff + N],
                        v5[:, kt, lh >> 1, lh & 1, :],
                        p[:, off:off + N],
                        start=(kt == 0), stop=(kt == ctx["last_kt"]),
                        skip_group_check=True)

            def finalize_pair(A, Bc):
                # A: po=0, denom rows 64:128 ; B: po=64, denom rows 0:64.
                # Pack both denominators into one [128,512] reciprocal
                # (no partition shift: B's half at rows 0:64, A's at 64:128).
                puA, puB = A["pu"], Bc["pu"]
                rin = wp.tile([128, 512], F32, tag="den", bufs=3, name="rin")
                nc.vector.tensor_copy(rin[0:64, :], puB[0:64, :])
                nc.vector.tensor_copy(rin[64:128, :], puA[64:128, :])
                rec = wp.tile([128, 512], F32, tag="rec", bufs=3, name="rec")
                nc.vector.reciprocal_approx_fast(rec[:, :], rin[:, :])
                nc.vector.tensor_mul(
                    aT[0:64, A["fbq"], A["qb"]:A["qb"] + 512],
                    puA[0:64, :], rec[64:128, :])
                nc.vector.tensor_mul(
                    aT[64:128, Bc["fbq"], Bc["qb"]:Bc["qb"] + 512],
                    puB[64:128, :], rec[0:64, :])

            def finalize_single(ctx):
                pu = ctx["pu"]
                po = ctx["po"]
                db = 64 - po             # denominator partitions
                den = wp.tile([64, 512], F32, tag="den", bufs=3, name="den")
                nc.vector.tensor_copy(den[:, :], pu[db:db + 64, :])
                rec = wp.tile([64, 512], F32, tag="rec", bufs=3, name="rec")
                nc.vector.reciprocal_approx_fast(rec[:, :], den[:, :])
                nc.vector.tensor_mul(
                    aT[po:po + 64, ctx["fbq"], ctx["qb"]:ctx["qb"] + 512],
                    pu[po:po + 64, :], rec[:, :])

            pending = []

            def flush_pending():
                while pending:
                    pending.pop(0)()

            # ---- stage 4: c_proj partial from a^T ----
            store_ring = [nc.sync, nc.gpsimd]
            late_ring = [nc.sync, nc.gpsimd, nc.scalar]
            store_n = [0]

            def emit_proj(st):
                for n0 in range(0, NX, 512):
                    ps = psU.tile([128, 512], F32, tag="pu")
                    for kt in range(FL // 128):
                        nc.tensor.matmul(ps[:],
                                         aT[:, kt, st * 128:(st + 1) * 128],
                                         wproj[:, kt, n0:n0 + 512],
                                         start=(kt == 0),
                                         stop=(NB and kt == FL // 128 - 1))
                    if not NB:
                        nc.tensor.matmul(ps[:], ones_row[:],
                                         bp_row[:, n0:n0 + 512],
                                         start=False, stop=True)
                    dst = out_ext.ap()[st * 128:(st + 1) * 128, n0:n0 + 512]
                    osb = wp.tile([128, 512], BF16, tag="osb")
                    if st >= 12 and store_n[0] % 2 == 0:
                        nc.scalar.copy(osb[:], ps[:])
                    else:
                        nc.vector.tensor_copy(osb[:], ps[:])
                    rings = late_ring if st >= 12 else store_ring
                    ring = rings[store_n[0] % len(rings)]
                    store_n[0] += 1
                    ring.dma_start(out=dst, in_=osb[:])

            def emit_unit(u):
                kind = u[0]
                if kind == "v":
                    emit_v(u[1])
                elif kind == "qk":
                    emit_qk(u[1], u[2])
                else:
                    emit_proj(u[1])

            def emit_pair(pr, qc, fills=()):
                """fills: per-iteration lists of units, emitted BEFORE the
                next scores group so the in-order PE has independent work
                while the exp engine drains the score PSUM ring."""
                A = head_ctx(2 * pr, qc)
                Bc = head_ctx(2 * pr + 1, qc)
                n = len(A["groups"])
                # spread the fill lists evenly over iterations 0..n-2
                given = [list(f) for f in fills]
                fills = [[] for _ in range(n)]
                for idx, f in enumerate(given):
                    pos = min(n - 2, (idx * max(n - 1, 1)) // max(len(given), 1)) \
                        if n > 1 else 0
                    fills[pos].extend(f)
                emit_scores_pair(A, Bc, 0)
                flush_pending()
                for i in range(n):
                    if i < len(fills):
                        for u in fills[i]:
                            emit_unit(u)
                    if i + 1 < n:
                        emit_scores_pair(A, Bc, i + 1)
                    if i < n - 1:
                        emit_u(A, i)
                        emit_u(Bc, i)
                    else:
                        def tail(Ax=A, Bx=Bc, gi=i):
                            emit_u(Ax, gi)
                            emit_u(Bx, gi)
                            if DPK:
                                finalize_single(Ax)
                                finalize_single(Bx)
                            else:
                                finalize_pair(Ax, Bx)
                        pending.append(tail)
                for f in fills[n:]:
                    for u in f:
                        emit_unit(u)

            # ---- emission schedule ----
            # Pre-units before a pair = its qk-chunk dependencies; they
            # double as the PE fill while the previous pair's tail drains.
            emit_qk(0, 0)
            emit_qk(FB, 0)
            emit_v(0)
            emit_v(1)
            emit_pair(0, 0, [[("v", 2), ("v", 3)],
                             [("qk", 0, 512), ("qk", FB, 512)]])
            emit_pair(0, 1, [[("v", 4), ("v", 5)], [("v", 6), ("v", 7)],
                             [("qk", 1, 0), ("qk", FB + 1, 0)]])
            emit_pair(1, 0, [[("qk", 1, 512)], [("qk", FB + 1, 512)]])
            emit_pair(1, 1, [[("qk", 0, 1024)], [("qk", FB, 1024)],
                             [("v", 8), ("v", 9)]])
            emit_pair(0, 2, [[("v", 10), ("v", 11)], [("qk", 2, 0)],
                             [("qk", FB + 2, 0)]])
            emit_pair(2, 0, [[("qk", 2, 512)], [("qk", FB + 2, 512)]])
            emit_pair(2, 1, [[("qk", 1, 1024)], [("qk", FB + 1, 1024)],
                             [("v", 12)]])
            emit_pair(1, 2, [[("qk", 3, 0)], [("qk", FB + 3, 0)]])
            emit_pair(3, 0, [[("qk", 3, 512)], [("qk", FB + 3, 512)]])
            emit_pair(3, 1, [[("qk", 0, 1536)], [("qk", FB, 1536)],
                             [("v", 13), ("v", 14)]])
            emit_pair(0, 3, [[("v", 15)], [("qk", 2, 1024)],
                             [("qk", FB + 2, 1024)]])
            emit_pair(2, 2, [[("proj", 0)], [("qk", 1, 1536)],
                             [("qk", FB + 1, 1536)]])
            emit_pair(1, 3, [[("proj", 1)], [("proj", 2)],
                             [("qk", 3, 1024)], [("qk", FB + 3, 1024)]])
            emit_pair(3, 2, [[("proj", 3)], [("qk", 2, 1536)],
                             [("qk", FB + 2, 1536)]])
            emit_pair(2, 3, [[("proj", 4)], [("proj", 5)], [("proj", 8)],
                             [("proj", 9)], [("qk", 3, 1536)],
                             [("qk", FB + 3, 1536)]])
            emit_pair(3, 3, [[("proj", 6)], [("proj", 7)], [("proj", 10)],
                             [("proj", 11)]])
            flush_pending()
            for st in (12, 13, 14, 15):
                emit_proj(st)

    nc.compile()
    return nc


@functools.lru_cache(maxsize=2)
def _built(cfg: str):
    return _build(cfg)


def _in_maps(x, c_attn_w, c_attn_b, c_proj_w, c_proj_b):
    maps = []
    for core in range(N_CORES):
        b, hg = core // 2, core % 2
        f0 = hg * FL
        w_q = c_attn_w[:, f0:f0 + FL]
        w_k = c_attn_w[:, NX + f0:NX + f0 + FL]
        w_v = c_attn_w[:, 2 * NX + f0:2 * NX + f0 + FL]
        b_q = c_attn_b[f0:f0 + FL]
        b_k = c_attn_b[NX + f0:NX + f0 + FL]
        b_v = c_attn_b[2 * NX + f0:2 * NX + f0 + FL]
        w_qk = np.concatenate([w_q, w_k], axis=1).astype(BF)
        # [2FB, NX, 128]: per-fb contiguous slices for the startup DMAs
        w_qk8 = np.ascontiguousarray(
            w_qk.reshape(NX, 2 * FB, 128).transpose(1, 0, 2))
        xT_np = np.ascontiguousarray(x[b].T).astype(BF)
        x8 = np.ascontiguousarray(
            xT_np.reshape(NX, 8, 256).transpose(1, 0, 2))
        maps.append({
            "x8": x8,
            "w_qk8": w_qk8,
            "w_v": np.ascontiguousarray(w_v).astype(BF),
            "w_proj": np.ascontiguousarray(c_proj_w[f0:f0 + FL, :]).astype(BF),
            "b_qk": np.concatenate([b_q, b_k]).astype(np.float32)
                      .reshape(-1, 1),
            "bv_row": b_v.astype(BF).reshape(1, FL),
            "bp_row": (c_proj_b / 2.0).astype(BF).reshape(1, NX),
        })
    return maps


def _run(inputs, cfg=None, trace=False):
    if cfg is None:
        zero_bias = (not inputs["c_attn_b"].any()) and \
                    (not inputs["c_proj_b"].any())
        cfg = DEFAULT_CFG if zero_bias else DEFAULT_CFG_BIAS
    nc = _built(cfg)
    maps = _in_maps(inputs["x"], inputs["c_attn_w"], inputs["c_attn_b"],
                    inputs["c_proj_w"], inputs["c_proj_b"])
    res = run_bass_kernel_spmd(nc, maps, core_ids=list(range(N_CORES)),
                               trace=trace)
    out = np.empty((B, S, NX), dtype=np.float32)
    for b in range(B):
        out[b] = (res.results[2 * b]["out"].astype(np.float32) +
                  res.results[2 * b + 1]["out"].astype(np.float32))
    return out, res


def kernel(**inputs):
    out, _ = _run({k: np.asarray(v) for k, v in inputs.items()})
    return out


# revision 22
# speedup vs baseline: 1.0577x; 1.0577x over previous
"""Causal multi-head attention block (B=4, S=2048, NX=1024, H=16, D=64)
distributed over 8 TRN2 NeuronCores.

Sharding: core i handles batch b = i//2 and head-group hg = i%2 (8 of 16
heads).  Each core computes qkv for its heads, causal attention, and a
partial c_proj over its 512 feature rows; the per-batch pair of partials
is summed on the host while unsharding.

All matmuls run in bf16 (f32 PSUM accumulate).  Scores are computed in the
transposed orientation s^T[k, q] = k @ q^T as concurrent 64-row quadrant
pairs (head A rows 0:63, head B rows 64:127 -> full moving-bus rate).  The
u = p @ v stage runs transposed with a v|ones augmented stationary so the
softmax denominator accumulates in the same pass.

Schedule: dense fill units (qk chunks / v tiles / proj tiles) are emitted
BEFORE each scores group so the in-order PE never head-of-line blocks on
the exp (scalar) engine draining score PSUM buffers.  Input DMAs are
sliced so the first qk chunk depends on ~1.3 MB instead of 5 MB.
"""
import sys

sys.path.insert(0, "/opt/trn_rl_repo")

import functools

import ml_dtypes
import numpy as np

from concourse import bacc, mybir, tile
from concourse.bass_utils import run_bass_kernel_spmd

B, S, NX = 4, 2048, 1024
H, D = 16, 64
N_CORES = 8
HL = H // 2          # heads per core (local)
FL = HL * D          # local head feature width (512)
BF16 = mybir.dt.bfloat16
F32 = mybir.dt.float32
BF = ml_dtypes.bfloat16

NK = S // 128        # 16 k-tiles of 128
KK = NX // 128       # 8 contraction blocks
FB = FL // 128       # 4 feature blocks per q/k half

DEFAULT_CFG = "host-psw1024-psb3-pb8-pub2-nb"
DEFAULT_CFG_BIAS = "host-psw1024-psb3-pb8-pub2"


def _parse_cfg(cfg: str):
    parts = cfg.split("-")
    d = {"mode": parts[0], "psw": 1024, "psb": 3, "pb": 8, "pub": 2,
         "nb": False, "gtri": False, "dpk": False}
    for p in parts[1:]:
        if p.startswith("psw"):
            d["psw"] = int(p[3:])
        elif p.startswith("psb"):
            d["psb"] = int(p[3:])
        elif p.startswith("pub"):
            d["pub"] = int(p[3:])
        elif p.startswith("pb"):
            d["pb"] = int(p[2:])
        elif p in d:
            d[p] = True
    return d


def _build(cfg: str):
    c = _parse_cfg(cfg)
    PSW, PSB, PB, PUB = c["psw"], c["psb"], c["pb"], c["pub"]
    NB = c["nb"]
    GTRI = c["gtri"]     # move tri-mask muls to gpsimd
    DPK = c["dpk"]       # disable packed pair reciprocal
    GK = PSW // 512      # full k-tiles per exp group
    nc = bacc.Bacc("TRN2", target_bir_lowering=False, debug=False,
                   num_devices=N_CORES)

    xT_ext = nc.dram_tensor("x8", [8, NX, 256], BF16, kind="ExternalInput")
    wqk_ext = nc.dram_tensor("w_qk8", [2 * FB, NX, 128], BF16,
                             kind="ExternalInput")
    wv_ext = nc.dram_tensor("w_v", [NX, FL], BF16, kind="ExternalInput")
    wp_ext = nc.dram_tensor("w_proj", [FL, NX], BF16, kind="ExternalInput")
    bqk_ext = nc.dram_tensor("b_qk", [2 * FL, 1], F32, kind="ExternalInput")
    bv_ext = nc.dram_tensor("bv_row", [1, FL], BF16, kind="ExternalInput")
    bp_ext = nc.dram_tensor("bp_row", [1, NX], BF16, kind="ExternalInput")
    out_ext = nc.dram_tensor("out", [S, NX], BF16, kind="ExternalOutput")

    with tile.TileContext(nc) as tc:
        with tc.tile_pool(name="const", bufs=1) as cp, \
             tc.tile_pool(name="work", bufs=3) as wp, \
             tc.tile_pool(name="psS", bufs=PSB, space="PSUM") as psS, \
             tc.tile_pool(name="psU", bufs=PUB, space="PSUM") as psU:

            # ---- persistent SBUF tensors ----
            xT = cp.tile([128, KK, S], BF16, tag="xT")
            wqk = cp.tile([128, KK, 2 * FL], BF16, tag="wqk")
            wv = cp.tile([128, KK, FL], BF16, tag="wv")
            wproj = cp.tile([128, FL // 128, NX], BF16, tag="wproj")
            qkT = cp.tile([128, 2 * FL // 128, S], BF16, tag="qkT")
            # v_aug[k, kt, hh, par, 0:128]: per head pair hh, parity par:
            #   par=0 (even head): cols 0:64 = v, 64:128 = 1.0
            #   par=1 (odd head):  cols 0:64 = 1.0, 64:128 = v
            v5 = cp.tile([128, NK, HL // 2, 2, 128], BF16, tag="v5")
            aT = cp.tile([128, FL // 128, S], BF16, tag="aT")  # a^T [feat, q]
            bqk = cp.tile([128, 2 * FL // 128], F32, tag="bqk")
            bv_row = cp.tile([1, FL], BF16, tag="bv")
            bp_row = cp.tile([1, NX], BF16, tag="bp")
            ones_row = cp.tile([1, 128], BF16, tag="ones")
            tri = cp.tile([128, 128], BF16, tag="tri")

            # ---- input DMAs: priority-ordered contiguous slices.
            # Never on the scalar ring (DGE would steal exp-engine time).
            xs = [xT_ext.ap()[j].rearrange("(kk p) c -> p kk c", p=128)
                  for j in range(8)]
            wqk8 = [wqk_ext.ap()[fb].rearrange("(kk p) c -> p kk c", p=128)
                    for fb in range(2 * FB)]

            def dma_x(ring, j):
                ring.dma_start(out=xT[:, :, j * 256:(j + 1) * 256],
                               in_=xs[j])

            def dma_wqk(ring, fb):
                ring.dma_start(out=wqk[:, :, fb * 128:(fb + 1) * 128],
                               in_=wqk8[fb])

            dma_x(nc.sync, 0)
            dma_wqk(nc.gpsimd, 0)
            dma_wqk(nc.gpsimd, FB)
            dma_x(nc.sync, 1)
            nc.sync.dma_start(
                out=bqk[:, :],
                in_=bqk_ext.ap().rearrange("(fb p) o -> p (fb o)", p=128))
            nc.sync.dma_start(out=bv_row[:], in_=bv_ext.ap())
            nc.sync.dma_start(out=bp_row[:], in_=bp_ext.ap())
            nc.gpsimd.dma_start(
                out=wv[:, :, :],
                in_=wv_ext.ap().rearrange("(kk p) f -> p kk f", p=128))
            dma_x(nc.sync, 2)
            dma_x(nc.sync, 3)
            dma_wqk(nc.gpsimd, 1)
            dma_wqk(nc.gpsimd, FB + 1)
            dma_x(nc.sync, 4)
            dma_x(nc.sync, 5)
            dma_wqk(nc.gpsimd, 2)
            dma_wqk(nc.gpsimd, FB + 2)
            dma_x(nc.sync, 6)
            dma_x(nc.sync, 7)
            dma_wqk(nc.gpsimd, 3)
            dma_wqk(nc.gpsimd, FB + 3)
            nc.sync.dma_start(
                out=wproj[:, :, :],
                in_=wp_ext.ap().rearrange("(kt p) f -> p kt f", p=128))

            nc.vector.memset(ones_row[:], 1.0)
            # tri[p, f] = 1 if p <= f else 0 (keep-in on p > f, else fill 1)
            nc.vector.memset(tri[:], 0.0)
            nc.gpsimd.affine_select(
                out=tri[:], in_=tri[:],
                compare_op=mybir.AluOpType.is_gt,
                fill=1.0, base=0, pattern=[[-1, 128]], channel_multiplier=1,
            )
            # ones halves of v_aug (parity-swapped)
            nc.vector.memset(v5[:, :, :, 0, 64:128], 1.0)
            nc.vector.memset(v5[:, :, :, 1, 0:64], 1.0)

            # ---- stage 2: v (natural layout, split by head parity) ----
            def emit_v(st):
                ps = psS.tile([128, FL], F32, tag="ps")
                for kk in range(KK):
                    nc.tensor.matmul(ps[:], xT[:, kk, st * 128:(st + 1) * 128],
                                     wv[:, kk, :], start=(kk == 0),
                                     stop=(NB and kk == KK - 1))
                if not NB:
                    nc.tensor.matmul(ps[:], ones_row[:], bv_row[:],
                                     start=False, stop=True)
                ps_r = ps[:].rearrange("p (hh par d) -> p hh par d",
                                       par=2, d=D)
                nc.vector.tensor_copy(v5[:, st, :, 0, 0:D], ps_r[:, :, 0, :])
                nc.vector.tensor_copy(v5[:, st, :, 1, D:128], ps_r[:, :, 1, :])

            # ---- stage 1: q^T / k^T chunks (feature-major, 512 cols) ----
            def emit_qk(fb, c0):
                ps = psS.tile([128, 512], F32, tag="ps")
                for kk in range(KK):
                    nc.tensor.matmul(
                        ps[:], wqk[:, kk, fb * 128:(fb + 1) * 128],
                        xT[:, kk, c0:c0 + 512],
                        start=(kk == 0), stop=(kk == KK - 1))
                nc.vector.tensor_scalar_add(qkT[:, fb, c0:c0 + 512],
                                            ps[:], bqk[:, fb:fb + 1])

            # ---- stage 3: attention ----
            def head_ctx(lh, qc):
                n_full = 4 * qc
                groups = []
                kt0 = 0
                while kt0 < n_full:
                    g = min(GK, n_full - kt0)
                    groups.append([(kt0 + j, j * 512, 512, 0)
                                   for j in range(g)])
                    kt0 += g
                if PSW >= 1536:
                    diag_offs = (0, 512, 1024, 1280)
                    groups.append([(n_full + j, diag_offs[j], 512 - 128 * j,
                                    128 * j) for j in range(4)])
                else:
                    groups.append([(n_full + 0, 0, 512, 0),
                                   (n_full + 1, 512, 384, 128)])
                    groups.append([(n_full + 2, 0, 256, 256),
                                   (n_full + 3, 256, 128, 384)])
                return {"lh": lh, "fbq": lh // 2, "fbk": FB + lh // 2,
                        "po": (lh % 2) * 64, "qb": qc * 512, "qc": qc,
                        "n_full": n_full, "groups": groups,
                        "p": [None] * len(groups), "pu": None,
                        "last_kt": n_full + 3}

            def emit_scores_pair(A, Bc, gi):
                """Scores for both heads of a pair, one k-tile group, as
                concurrent 64-row quadrant matmuls (A rows 0:63, B 64:127)."""
                g = A["groups"][gi]
                gw = max(off + N for (_, off, N, _) in g)
                psA = psS.tile([128, PSW], F32, tag="ps", name="psA")
                psB = psS.tile([128, PSW], F32, tag="ps", name="psB")
                pA = wp.tile([128, PSW], BF16, tag="p", bufs=PB, name="pA")
                pB = wp.tile([128, PSW], BF16, tag="p", bufs=PB, name="pB")
                for (kt, off, N, qoff) in g:
                    k0 = kt * 128
                    amv = qkT[0:64, A["fbq"], A["qb"] + qoff:A["qb"] + 512]
                    bmv = qkT[64:128, Bc["fbq"],
                              Bc["qb"] + qoff:Bc["qb"] + 512]
                    nc.tensor.matmul(psA[:, off:off + N],
                                     qkT[0:64, A["fbk"], k0:k0 + 128],
                                     amv, start=True, stop=True)
                    nc.tensor.matmul(psB[:, off:off + N],
                                     qkT[64:128, Bc["fbk"], k0:k0 + 128],
                                     bmv, start=True, stop=True)
                for ctx, ps, p in ((A, psA, pA), (Bc, psB, pB)):
                    nc.scalar.activation(p[:, 0:gw], ps[:, 0:gw],
                                         mybir.ActivationFunctionType.Exp,
                                         scale=0.125)
                    if g[0][0] >= A["n_full"]:
                        eng = nc.gpsimd if GTRI else nc.vector
                        for (kt, off, N, qoff) in g:
                            eng.tensor_mul(p[:, off:off + 128],
                                           p[:, off:off + 128], tri)
                    ctx["p"][gi] = p

            def emit_u(ctx, gi):
                if ctx["pu"] is None:
                    ctx["pu"] = psU.tile([128, 512], F32, tag="pu",
                                         name="pu_t")
                pu = ctx["pu"]
                p = ctx["p"][gi]
                lh = ctx["lh"]
                for (kt, off, N, qoff) in ctx["groups"][gi]:
                    nc.tensor.matmul(
                        pu[:, qoff:qoff + N],
                        v5[:, kt, lh >> 1, lh & 1, :],
                        p[:, off:off + N],
                        start=(kt == 0), stop=(kt == ctx["last_kt"]),
                        skip_group_check=True)

            def finalize_pair(A, Bc):
                # A: po=0, denom rows 64:128 ; B: po=64, denom rows 0:64.
                # Pack both denominators into one [128,512] reciprocal
                # (no partition shift: B's half at rows 0:64, A's at 64:128).
                puA, puB = A["pu"], Bc["pu"]
                rin = wp.tile([128, 512], F32, tag="den", bufs=3, name="rin")
                nc.vector.tensor_copy(rin[0:64, :], puB[0:64, :])
                nc.vector.tensor_copy(rin[64:128, :], puA[64:128, :])
                rec = wp.tile([128, 512], F32, tag="rec", bufs=3, name="rec")
                nc.vector.reciprocal_approx_fast(rec[:, :], rin[:, :])
                nc.vector.tensor_mul(
                    aT[0:64, A["fbq"], A["qb"]:A["qb"] + 512],
                    puA[0:64, :], rec[64:128, :])
                nc.vector.tensor_mul(
                    aT[64:128, Bc["fbq"], Bc["qb"]:Bc["qb"] + 512],
                    puB[64:128, :], rec[0:64, :])

            def finalize_single(ctx):
                pu = ctx["pu"]
                po = ctx["po"]
                db = 64 - po             # denominator partitions
                den = wp.tile([64, 512], F32, tag="den", bufs=3, name="den")
                nc.vector.tensor_copy(den[:, :], pu[db:db + 64, :])
                rec = wp.tile([64, 512], F32, tag="rec", bufs=3, name="rec")
                nc.vector.reciprocal_approx_fast(rec[:, :], den[:, :])
                nc.vector.tensor_mul(
                    aT[po:po + 64, ctx["fbq"], ctx["qb"]:ctx["qb"] + 512],
                    pu[po:po + 64, :], rec[:, :])

            pending = []

            def flush_pending():
                while pending:
                    pending.pop(0)()

            # ---- stage 4: c_proj partial from a^T ----
            store_ring = [nc.sync, nc.gpsimd]
            late_ring = [nc.sync, nc.gpsimd, nc.scalar]
            store_n = [0]

            def emit_proj(st):
                for n0 in range(0, NX, 512):
                    ps = psU.tile([128, 512], F32, tag="pu")
                    for kt in range(FL // 128):
                        nc.tensor.matmul(ps[:],
                                         aT[:, kt, st * 128:(st + 1) * 128],
                                         wproj[:, kt, n0:n0 + 512],
                                         start=(kt == 0),
                                         stop=(NB and kt == FL // 128 - 1))
                    if not NB:
                        nc.tensor.matmul(ps[:], ones_row[:],
                                         bp_row[:, n0:n0 + 512],
                                         start=False, stop=True)
                    dst = out_ext.ap()[st * 128:(st + 1) * 128, n0:n0 + 512]
                    osb = wp.tile([128, 512], BF16, tag="osb")
                    if st >= 12 and store_n[0] % 2 == 0:
                        nc.scalar.copy(osb[:], ps[:])
                    else:
                        nc.vector.tensor_copy(osb[:], ps[:])
                    rings = late_ring if st >= 12 else store_ring
                    ring = rings[store_n[0] % len(rings)]
                    store_n[0] += 1
                    ring.dma_start(out=dst, in_=osb[:])

            def emit_unit(u):
                kind = u[0]
                if kind == "v":
                    emit_v(u[1])
                elif kind == "qk":
                    emit_qk(u[1], u[2])
                else:
                    emit_proj(u[1])

            def emit_pair(pr, qc, fills=()):
                """fills: per-iteration lists of units, emitted BEFORE the
                next scores group so the in-order PE has independent work
                while the exp engine drains the score PSUM ring."""
                A = head_ctx(2 * pr, qc)
                Bc = head_ctx(2 * pr + 1, qc)
                fills = [list(f) for f in fills]
                n = len(A["groups"])
                emit_scores_pair(A, Bc, 0)
                flush_pending()
                for i in range(n):
                    if i < len(fills):
                        for u in fills[i]:
                            emit_unit(u)
                    if i + 1 < n:
                        emit_scores_pair(A, Bc, i + 1)
                    if i < n - 1:
                        emit_u(A, i)
                        emit_u(Bc, i)
                    else:
                        def tail(Ax=A, Bx=Bc, gi=i):
                            emit_u(Ax, gi)
                            emit_u(Bx, gi)
                            if DPK:
                                finalize_single(Ax)
                                finalize_single(Bx)
                            else:
                                finalize_pair(Ax, Bx)
                        pending.append(tail)
                for f in fills[n:]:
                    for u in f:
                        emit_unit(u)

            # ---- emission schedule ----
            # Pre-units before a pair = its qk-chunk dependencies; they
            # double as the PE fill while the previous pair's tail drains.
            emit_qk(0, 0)
            emit_qk(FB, 0)
            emit_v(0)
            emit_v(1)
            emit_pair(0, 0, [[("v", 2), ("v", 3)],
                             [("qk", 0, 512), ("qk", FB, 512)]])
            emit_pair(0, 1, [[("v", 4), ("v", 5)], [("v", 6), ("v", 7)],
                             [("qk", 1, 0), ("qk", FB + 1, 0)]])
            emit_pair(1, 0, [[("qk", 1, 512)], [("qk", FB + 1, 512)]])
            emit_pair(1, 1, [[("qk", 0, 1024)], [("qk", FB, 1024)],
                             [("v", 8), ("v", 9)]])
            emit_pair(0, 2, [[("v", 10), ("v", 11)], [("qk", 2, 0)],
                             [("qk", FB + 2, 0)]])
            emit_pair(2, 0, [[("qk", 2, 512)], [("qk", FB + 2, 512)]])
            emit_pair(2, 1, [[("qk", 1, 1024)], [("qk", FB + 1, 1024)],
                             [("v", 12)]])
            emit_pair(1, 2, [[("qk", 3, 0)], [("qk", FB + 3, 0)]])
            emit_pair(3, 0, [[("qk", 3, 512)], [("qk", FB + 3, 512)]])
            emit_pair(3, 1, [[("qk", 0, 1536)], [("qk", FB, 1536)],
                             [("v", 13), ("v", 14)]])
            emit_pair(0, 3, [[("v", 15)], [("qk", 2, 1024)],
                             [("qk", FB + 2, 1024)]])
            emit_pair(2, 2, [[("proj", 0)], [("qk", 1, 1536)],
                             [("qk", FB + 1, 1536)]])
            emit_pair(1, 3, [[("proj", 1)], [("proj", 2)],
                             [("qk", 3, 1024)], [("qk", FB + 3, 1024)]])
            emit_pair(3, 2, [[("proj", 3)], [("qk", 2, 1536)],
                             [("qk", FB + 2, 1536)]])
            emit_pair(2, 3, [[("proj", 4)], [("proj", 5)], [("proj", 8)],
                             [("proj", 9)], [("qk", 3, 1536)],
                             [("qk", FB + 3, 1536)]])
            emit_pair(3, 3, [[("proj", 6)], [("proj", 7)], [("proj", 10)],
                             [("proj", 11)]])
            flush_pending()
            for st in (12, 13, 14, 15):
                emit_proj(st)

    nc.compile()
    return nc


@functools.lru_cache(maxsize=2)
def _built(cfg: str):
    return _build(cfg)


def _in_maps(x, c_attn_w, c_attn_b, c_proj_w, c_proj_b):
    maps = []
    for core in range(N_CORES):
        b, hg = core // 2, core % 2
        f0 = hg * FL
        w_q = c_attn_w[:, f0:f0 + FL]
        w_k = c_attn_w[:, NX + f0:NX + f0 + FL]
        w_v = c_attn_w[:, 2 * NX + f0:2 * NX + f0 + FL]
        b_q = c_attn_b[f0:f0 + FL]
        b_k = c_attn_b[NX + f0:NX + f0 + FL]
        b_v = c_attn_b[2 * NX + f0:2 * NX + f0 + FL]
        w_qk = np.concatenate([w_q, w_k], axis=1).astype(BF)
        # [2FB, NX, 128]: per-fb contiguous slices for the startup DMAs
        w_qk8 = np.ascontiguousarray(
            w_qk.reshape(NX, 2 * FB, 128).transpose(1, 0, 2))
        xT_np = np.ascontiguousarray(x[b].T).astype(BF)
        x8 = np.ascontiguousarray(
            xT_np.reshape(NX, 8, 256).transpose(1, 0, 2))
        maps.append({
            "x8": x8,
            "w_qk8": w_qk8,
            "w_v": np.ascontiguousarray(w_v).astype(BF),
            "w_proj": np.ascontiguousarray(c_proj_w[f0:f0 + FL, :]).astype(BF),
            "b_qk": np.concatenate([b_q, b_k]).astype(np.float32)
                      .reshape(-1, 1),
            "bv_row": b_v.astype(BF).reshape(1, FL),
            "bp_row": (c_proj_b / 2.0).astype(BF).reshape(1, NX),
        })
    return maps


def _run(inputs, cfg=None, trace=False):
    if cfg is None:
        zero_bias = (not inputs["c_attn_b"].any()) and \
                    (not inputs["c_proj_b"].any())
        cfg = DEFAULT_CFG if zero_bias else DEFAULT_CFG_BIAS
    nc = _built(cfg)
    maps = _in_maps(inputs["x"], inputs["c_attn_w"], inputs["c_attn_b"],
                    inputs["c_proj_w"], inputs["c_proj_b"])
    res = run_bass_kernel_spmd(nc, maps, core_ids=list(range(N_CORES)),
                               trace=trace)
    out = np.empty((B, S, NX), dtype=np.float32)
    for b in range(B):
        out[b] = (res.results[2 * b]["out"].astype(np.float32) +
                  res.results[2 * b + 1]["out"].astype(np.float32))
    return out, res


def kernel(**inputs):
    out, _ = _run({k: np.asarray(v) for k, v in inputs.items()})
    return out
